# revision 11
# baseline (speedup 1.0000x reference)
"""BinConv3d (sign-binarized 3x3x3 conv, NCDHW) on 8 Trainium2 NeuronCores.

Full inputs in, full output out. Sharding: depth dim D=64 split 8 ways
(8 output planes per core) with a 1-plane halo on the input; conv weights
and bias replicated to every core.

Host prep: each core's input slab is rearranged to [plane, quarter, ci,
34, 130] fp32 — the H dim split into 4 quarter-row panels of 32 rows,
each padded with 1 halo row above/below and 1 zero col left/right, so
the device needs no data reshuffling at all.

Per-core kernel (Bass/Tile):
  - One [128, 34*130] DMA per plane (partition = quarter*32 + ci), then
    ScalarE Sign fp32 -> bf16 (zero pads stay zero).
  - Conv = 27 accumulating matmuls (K=32 ci, M=64 co, N=512) per 4-row
    output tile; every (kd, kh, kw) tap is a free-dim AP offset.
  - 16-way PE tiling: quarter q runs on PE row-group q (tile_position
    row 32q); even/odd 4-row blocks run on PE column halves. 8 matmuls
    issue back-to-back per tap and run concurrently: full 128x128 array.
  - PSUM: 4 banks per generation (bank = quarter, partitions 0-63 even
    block / 64-127 odd block), double-buffered = all 8 banks.
  - PSUM drained with bias add on ScalarE (even) / VectorE (odd) into a
    [128, 2048] staging tile, flushed to HBM as 2x512KB DMAs on
    complementary SBUF port sets.
"""

import numpy as np
import ml_dtypes

import concourse.bass as bass
import concourse.mybir as mybir
import concourse.tile as tile
from concourse import bacc
from concourse.bass import ts
from concourse.bass_utils import run_bass_kernel_spmd

CI = 32
CO = 64
D_FULL = 64
N_CORES = 8
D_OUT = D_FULL // N_CORES  # output planes per core
D_IN = D_OUT + 2  # input planes per core (1-plane halo each side)

_cache = {}


def build_conv_program(n_in_planes=D_IN, n_out_planes=D_OUT, H=128, W=128,
                       debug=False):
    """Build the per-core Bass program (SPMD: same program on all cores)."""
    f32 = mybir.dt.float32
    bf16 = mybir.dt.bfloat16
    Hq = H // 4          # rows per quarter-panel
    Hqp, Wp = Hq + 2, W + 2
    n_pairs = Hq // 8    # even/odd block pairs per quarter
    assert Hq % 8 == 0 and W == 128

    nc = bacc.Bacc("TRN2", target_bir_lowering=False, debug=debug)
    x_in = nc.declare_dram_parameter(
        "xs", [n_in_planes, 4, CI, Hqp, Wp], f32, isOutput=False)
    w_in = nc.declare_dram_parameter("wst", [128, 27, CO], bf16,
                                     isOutput=False)
    b_in = nc.declare_dram_parameter("bias", [128, 1], f32, isOutput=False)
    y_out = nc.declare_dram_parameter("y", [CO, n_out_planes, H, W], f32,
                                      isOutput=True)

    with tile.TileContext(nc) as tc:
        with (
            tc.tile_pool(name="const", bufs=1) as constp,
            tc.tile_pool(name="raw", bufs=2) as rawp,
            tc.tile_pool(name="sgn", bufs=4) as sgnp,
            tc.tile_pool(name="stg", bufs=3) as stgp,
            tc.tile_pool(name="psum", bufs=2, space="PSUM") as psump,
        ):
            wt = constp.tile([128, 27, CO], bf16)
            nc.sync.dma_start(out=wt[:], in_=w_in[:])
            bs = constp.tile([128, 1], f32)
            nc.sync.dma_start(out=bs[:], in_=b_in[:])

            sgns = {}

            def load_plane(p):
                raw = rawp.tile([128, Hqp, Wp], f32, tag="raw")
                nc.sync.dma_start(
                    out=raw[:],
                    in_=x_in[p].rearrange("q c h w -> (q c) h w"),
                )
                sgn = sgnp.tile([128, Hqp, Wp], bf16, tag="sgn")
                nc.scalar.sign(sgn[:], raw[:])
                sgns[p] = sgn

            for p in range(3):
                load_plane(p)

            ident = mybir.ActivationFunctionType.Identity
            for d in range(n_out_planes):
                if d + 3 < n_in_planes:
                    load_plane(d + 3)
                for pi in range(n_pairs):
                    # generation: for each quarter, blocks 2*pi (even,
                    # cols 0-63) and 2*pi+1 (odd, cols 64-127)
                    pts = [psump.tile([128, 512], f32, tag=f"pt{q}",
                                      name=f"pt{q}_{d}_{pi}")
                           for q in range(4)]
                    for tap in range(27):
                        kd, r = divmod(tap, 9)
                        kh, kw = divmod(r, 3)
                        sg = sgns[d + kd]
                        for q in range(4):
                            for half in range(2):
                                blk = 2 * pi + half
                                rhs = sg[32 * q:32 * q + 32,
                                         4 * blk + kh:4 * blk + kh + 4,
                                         kw:kw + W]
                                nc.tensor.matmul(
                                    pts[q][64 * half:64 * half + 64, :],
                                    lhsT=wt[32 * q:32 * q + 32, tap, :],
                                    rhs=rhs,
                                    start=(tap == 0),
                                    stop=(tap == 26),
                                    tile_position=(32 * q, 64 * half),
                                    skip_group_check=True,
                                )
                    # drain: stg[64*half+co, q*512 + rw]
                    stg = stgp.tile([128, 4 * 512], f32, tag="stg")
                    for q in range(4):
                        nc.scalar.activation(
                            stg[0:64, ts(q, 512)], pts[q][0:64, :], ident,
                            bias=bs[0:64], scale=1.0,
                        )
                        nc.vector.tensor_scalar_add(
                            out=stg[64:128, ts(q, 512)], in0=pts[q][64:128, :],
                            scalar1=bs[64:128],
                        )
                    # out rows: quarter q -> 32q + 8*pi + 4*half .. +4
                    yv = y_out[:, d].rearrange("co (q hi) w -> co q hi w", q=4)
                    for half in range(2):
                        dst = yv[:, :, 8 * pi + 4 * half:8 * pi + 4 * half + 4,
                                 :].rearrange("co q hi w -> co q (hi w)")
                        src = stg[64 * half:64 * half + 64, :].rearrange(
                            "co (q n) -> co q n", q=4)
                        nc.sync.dma_start(out=dst, in_=src)

    nc.compile()
    return nc


def _get_program():
    if "nc" not in _cache:
        _cache["nc"] = build_conv_program()
    return _cache["nc"]


def prep_weights(W, b):
    W = np.asarray(W, dtype=np.float32)
    b = np.asarray(b, dtype=np.float32)
    # wst[q*32+ci, kd*9+kh*3+kw, co] = W[co, ci, kd, kh, kw], replicated 4x
    wq = W.transpose(1, 2, 3, 4, 0).reshape(CI, 27, CO)
    wst = np.ascontiguousarray(
        np.broadcast_to(wq[None], (4, CI, 27, CO)).reshape(128, 27, CO)
    ).astype(ml_dtypes.bfloat16)
    bias = np.ascontiguousarray(
        np.concatenate([b, b]).reshape(128, 1).astype(np.float32))
    return wst, bias


def prep_x_slab(xpad, p_lo, n_planes, H=128, W=128):
    """xpad: [CI, D+2, H+2, W+2] zero-padded input. Returns
    [n_planes, 4, CI, H//4+2, W+2] fp32 slab for planes p_lo..p_lo+n_planes."""
    Hq = H // 4
    out = np.empty((n_planes, 4, CI, Hq + 2, W + 2), dtype=np.float32)
    for q in range(4):
        # padded rows 32q .. 32q+34 cover global rows 32q-1 .. 32q+33
        out[:, q] = xpad[:, p_lo:p_lo + n_planes,
                         Hq * q:Hq * q + Hq + 2, :].transpose(1, 0, 2, 3)
    return out


def _prep_inputs(x, W, b):
    x = np.asarray(x, dtype=np.float32)
    wst, bias = prep_weights(W, b)
    xpad = np.pad(x[0], ((0, 0), (1, 1), (1, 1), (1, 1)))
    in_maps = []
    for k in range(N_CORES):
        xs = prep_x_slab(xpad, D_OUT * k, D_IN)
        in_maps.append({"xs": xs, "wst": wst, "bias": bias})
    return in_maps


def run(x, W, b, trace=False):
    """Run the kernel; returns (output, BassKernelResults)."""
    nc = _get_program()
    in_maps = _prep_inputs(x, W, b)
    res = run_bass_kernel_spmd(nc, in_maps, list(range(N_CORES)), trace=trace)
    y = np.concatenate([res.results[k]["y"] for k in range(N_CORES)], axis=1)
    return y[None], res


def kernel(x, W, b):
    y, _ = run(x, W, b)
    return y
